# revision 7
# baseline (speedup 1.0000x reference)
"""Multi-head causal attention (B=4, S=2048, D=1024, H=16, HD=64) on 8 TRN2 cores.

Sharding: tensor-parallel on heads (4-way: 4 heads per core) x data-parallel
on batch (2-way: 2 batches per core).  Core c: tp = c % 4 owns head columns
[256*tp, 256*tp+256); dp = c // 4 owns batches {2*dp, 2*dp+1}.

Each core computes, per batch:
  QT = (x @ Wq_shard)^T  [256, 2048]   (via lhsT=Wq tiles, rhs=x^T tiles)
  KT = (x @ Wk_shard)^T  [256, 2048]
  V  =  x @ Wv_shard     [2048, 256]   (bf16, stored with interleaved ones col)
  per head pair: S^T = K @ Q^T tiles [128 kseq, 512+512 q]  (fp32r matmuls,
    2 heads ride concurrently via tile_position row packing)
  P^T = exp(S^T / 8)  (no max subtraction -- scores are O(1) bounded), bf16
  causal mask on diagonal blocks (precomputed bf16 masks, DVE multiply)
  ctx^T_aug [65, q] += [V_h | ones]^T @ P^T_h   (l rides as row 64)
  normalize: rec = 1/l (DVE approx), gpsimd partition-broadcast, DVE multiply
  out^T_partial = Wo_shard^T-tiles @ ctx^T  ->  DRAM [1024, 2048]

Host: out[b] = sum_tp outT + bo.
"""

import sys

sys.path.insert(0, "/opt/trn_rl_repo")

import numpy as np
import ml_dtypes

import concourse.bass as bass
import concourse.mybir as mybir
import concourse.tile as tile
from concourse import bacc
from concourse.bass_utils import run_bass_kernel_spmd

# ---------------------------------------------------------------- dimensions
B, S, D, H, HD = 4, 2048, 1024, 16, 64
NCORES = 8
TP, DP = 4, 2                 # head-parallel x batch-parallel
NB = B // DP                  # batches per core (2)
DHC = D // TP                 # head cols per core (256)
NHC = DHC // HD               # heads per core (4)
NPAIR = NHC // 2              # head pairs per core (2)
P = 128
CH = 512                      # q chunk width
NCH = S // CH                 # 4
KD = D // P                   # k tiles for qkv proj (8)
NKB = S // P                  # kseq blocks (16)
VW = HD + 2                   # per-head V block width (64 V + 1 ones + 1 pad)

f32 = mybir.dt.float32
f32r = mybir.dt.float32r
bf16 = mybir.dt.bfloat16

SCALE = 1.0 / np.sqrt(HD)

# dtype knobs
MM_DT = f32r                  # matmul compute dtype for X/W/QT/KT/ctx paths
PV_DT = bf16                  # P^T and V dtype (ctx matmul operands)


def _r(ap):
    """bitcast an f32 AP to the matmul compute dtype"""
    return ap.bitcast(MM_DT)


def _mask_np():
    """4 doubled diagonal masks, pattern k: valid iff c >= r + 128*k.
    Shape [128, 4*1024] bf16 (each mask [128, 512] doubled for 2 heads)."""
    r = np.arange(P)[:, None]
    c = np.arange(CH)[None, :]
    ms = []
    for k in range(4):
        m = (c >= r + P * k).astype(np.float32)
        ms.append(np.concatenate([m, m], axis=1))
    return np.concatenate(ms, axis=1).astype(ml_dtypes.bfloat16)


def build_program(nc):
    """Declare IO and emit the Tile program onto nc."""
    xT = nc.dram_tensor("xT", [NB, D, S], MM_DT, kind="ExternalInput")
    wq = nc.dram_tensor("wq", [D, DHC], MM_DT, kind="ExternalInput")
    wk = nc.dram_tensor("wk", [D, DHC], MM_DT, kind="ExternalInput")
    wv = nc.dram_tensor("wv", [D, DHC], MM_DT, kind="ExternalInput")
    wo = nc.dram_tensor("wo", [DHC, D], MM_DT, kind="ExternalInput")
    outT = nc.dram_tensor("outT", [NB, D, S], f32, kind="ExternalOutput")
    masks_dram = nc.inline_tensor(_mask_np(), name="masks")

    with tile.TileContext(nc) as tc:
        _emit(tc, xT.ap(), wq.ap(), wk.ap(), wv.ap(), wo.ap(), outT.ap(),
              masks_dram.ap())
    return nc


def _emit(tc, xT, wq, wk, wv, wo, outT, masks_dram):
    nc = tc.nc
    Exp = mybir.ActivationFunctionType.Exp
    mult = mybir.AluOpType.mult

    from contextlib import ExitStack
    ctx = ExitStack()
    with ctx:
        const_pool = ctx.enter_context(tc.tile_pool(name="const", bufs=1))
        w_pool = ctx.enter_context(tc.tile_pool(name="w", bufs=1))
        xt_pool = ctx.enter_context(tc.tile_pool(name="xt", bufs=8))
        qt_pool = ctx.enter_context(tc.tile_pool(name="qt", bufs=4))
        v_pool = ctx.enter_context(tc.tile_pool(name="v", bufs=18))
        ctx_pool = ctx.enter_context(tc.tile_pool(name="ctx", bufs=2))
        p_pool = ctx.enter_context(tc.tile_pool(name="pt", bufs=3))
        rec_pool = ctx.enter_context(tc.tile_pool(name="rec", bufs=2))
        l0_pool = ctx.enter_context(tc.tile_pool(name="l0", bufs=2))
        rc_pool = ctx.enter_context(tc.tile_pool(name="rc", bufs=2))
        bs_pool = ctx.enter_context(tc.tile_pool(name="bs", bufs=2))
        tmp_pool = ctx.enter_context(tc.tile_pool(name="tmpn", bufs=2))
        stage_pool = ctx.enter_context(tc.tile_pool(name="stage", bufs=4))
        psum_mm = ctx.enter_context(
            tc.tile_pool(name="psmm", bufs=3, space="PSUM"))
        psum_ctx = ctx.enter_context(
            tc.tile_pool(name="psctx", bufs=2, space="PSUM"))

        # ---- gpsimd ucode library for partition_broadcast
        from concourse import library_config
        nc.gpsimd.load_library(library_config.attn)

        # ---- constants + weights to SBUF
        mask_sb = const_pool.tile([P, 4 * 2 * CH], bf16)
        nc.sync.dma_start(mask_sb[:], masks_dram[:])

        wq_sb = w_pool.tile([P, KD, DHC], MM_DT)
        nc.sync.dma_start(wq_sb[:], wq.rearrange("(ko p) n -> p ko n", p=P))
        wk_sb = w_pool.tile([P, KD, DHC], MM_DT)
        nc.sync.dma_start(wk_sb[:], wk.rearrange("(ko p) n -> p ko n", p=P))
        wv_sb = w_pool.tile([P, KD, DHC], MM_DT)
        nc.sync.dma_start(wv_sb[:], wv.rearrange("(ko p) n -> p ko n", p=P))
        wo_sb = w_pool.tile([P, DHC // P, D], MM_DT)
        nc.sync.dma_start(wo_sb[:], wo.rearrange("(ko p) n -> p ko n", p=P))

        for b in range(NB):
            # ---- load x^T tiles
            xt_t = []
            x_src = xT[b].rearrange("(ko p) n -> ko p n", p=P)
            for k in range(KD):
                t = xt_pool.tile([P, S], MM_DT, tag="xt")
                nc.sync.dma_start(t[:], x_src[k])
                xt_t.append(t)

            # ---- V projection: V[m*128:, :DHC] with interleaved ones cols
            v_t = []
            for m in range(NKB):
                ps = psum_mm.tile([P, 2 * CH], f32, tag="mm", name="ps_v")[:, :DHC]
                for k in range(KD):
                    nc.tensor.matmul(
                        ps, xt_t[k][:, m * P:(m + 1) * P],
                        wv_sb[:, k],
                        start=(k == 0), stop=(k == KD - 1))
                vt = v_pool.tile([P, NHC * VW], PV_DT, tag="v")
                nc.vector.memset(vt[:], 1.0)
                nc.vector.tensor_copy(
                    vt.rearrange("p (h w) -> p h w", w=VW)[:, :, :HD],
                    ps.rearrange("p (h w) -> p h w", w=HD))
                v_t.append(vt)

            # ---- Q^T / K^T projections: [128, S] tiles per head pair
            qt_t, kt_t = [], []
            for (w_sb, dst) in ((wq_sb, qt_t), (wk_sb, kt_t)):
                for mt in range(NPAIR):
                    t = qt_pool.tile([P, S], MM_DT, tag="qt")
                    for chix in range(NCH):
                        ps = psum_mm.tile([P, 2 * CH], f32, tag="mm",
                                          name="ps_qk")[:, :CH]
                        for k in range(KD):
                            nc.tensor.matmul(
                                ps,
                                w_sb[:, k, mt * P:(mt + 1) * P],
                                xt_t[k][:, chix * CH:(chix + 1) * CH],
                                start=(k == 0), stop=(k == KD - 1))
                        nc.vector.tensor_copy(
                            t[:, chix * CH:(chix + 1) * CH], ps)
                    dst.append(t)

            # ---- attention + output ctx^T tiles
            ctx_t = []
            for pr in range(NPAIR):
                ct = ctx_pool.tile([P, S], MM_DT, tag="ctx")
                qt, kt = qt_t[pr], kt_t[pr]
                for j in range(NCH):
                    nblk = 4 * j + 4
                    a_ps = [psum_ctx.tile([HD + 1, CH], f32, tag="actx",
                                         name=f"actx{_h}")
                            for _h in range(2)]
                    pts = [None] * nblk
                    for i in range(nblk):
                        # scores S^T block [128 kseq, 512 q] x 2 heads
                        sc = psum_mm.tile([P, 2 * CH], f32, tag="mm")
                        for hh in range(2):
                            nc.tensor.matmul(
                                sc[:, hh * CH:(hh + 1) * CH],
                                kt[hh * HD:(hh + 1) * HD,
                                   i * P:(i + 1) * P],
                                qt[hh * HD:(hh + 1) * HD,
                                   j * CH:(j + 1) * CH],
                                start=True, stop=True,
                                tile_position=(hh * HD, 0))
                        pt = p_pool.tile([P, 2 * CH], PV_DT, tag="pt")
                        nc.scalar.activation(pt[:], sc[:], Exp, scale=SCALE)
                        kdiag = i - 4 * j
                        if kdiag >= 0:
                            nc.vector.tensor_tensor(
                                pt[:], pt[:],
                                mask_sb[:, kdiag * 2 * CH:(kdiag + 1) * 2 * CH],
                                mult)
                        pts[i] = pt
                        # software-pipeline: ctx for block i-1 after scores i
                        if i >= 1:
                            _ctx_mm(nc, a_ps, v_t[i - 1], pts[i - 1], pr,
                                    start=(i - 1 == 0), stop=False)
                            pts[i - 1] = None
                    _ctx_mm(nc, a_ps, v_t[nblk - 1], pts[nblk - 1], pr,
                            start=(nblk == 1), stop=True)

                    # normalize both heads of chunk j
                    # (l must reach partition 0: partition_broadcast and
                    # reciprocal_approx_fast misread APs based at row 64
                    # on HW, so copy PSUM row 64 out and DMA-shift to row 0)
                    for hh in range(2):
                        acc = a_ps[hh]
                        lrow = rec_pool.tile([HD + 1, CH], f32, tag="rec",
                                             name="lrow")
                        nc.vector.tensor_copy(
                            lrow[HD:HD + 1, :], acc[HD:HD + 1, :])
                        l0 = l0_pool.tile([1, CH], f32, tag="l0", name="l0")
                        nc.sync.dma_start(l0[:], lrow[HD:HD + 1, :])
                        rec = rc_pool.tile([1, CH], f32, tag="rc", name="rc")
                        nc.vector.reciprocal_approx_fast(rec[:], l0[:])
                        bsb = bs_pool.tile([HD, CH], f32, tag="bs")
                        nc.gpsimd.partition_broadcast(bsb[:], rec[:])
                        if hh == 0:
                            nc.vector.tensor_tensor(
                                ct[0:HD, j * CH:(j + 1) * CH],
                                acc[0:HD, :], bsb[:], mult)
                        else:
                            tmpn = tmp_pool.tile([HD, CH], MM_DT, tag="tmpn")
                            nc.vector.tensor_tensor(
                                tmpn[:], acc[0:HD, :], bsb[:], mult)
                            nc.sync.dma_start(
                                ct[HD:2 * HD, j * CH:(j + 1) * CH], tmpn[:])
                ctx_t.append(ct)

            # ---- out projection: out^T [1024, 2048] partial
            o_dst = outT[b].rearrange("(mo p) n -> mo p n", p=P)
            for mt in range(D // P):
                for chix in range(NCH):
                    ps = psum_mm.tile([P, 2 * CH], f32, tag="mm",
                                      name="ps_o")[:, :CH]
                    for kt_i in range(NPAIR):
                        nc.tensor.matmul(
                            ps,
                            wo_sb[:, kt_i, mt * P:(mt + 1) * P],
                            ctx_t[kt_i][:, chix * CH:(chix + 1) * CH],
                            start=(kt_i == 0), stop=(kt_i == NPAIR - 1))
                    st = stage_pool.tile([P, CH], f32, tag="stage")
                    nc.scalar.copy(st[:], ps)
                    nc.sync.dma_start(
                        o_dst[mt][:, chix * CH:(chix + 1) * CH], st[:])


def _ctx_mm(nc, a_ps, vt, pt, pr, start, stop):
    """ctx^T_aug[65, 512] += [V_h|1]^T @ P^T_h for both heads of pair pr."""
    vv = vt.rearrange("p (h w) -> p h w", w=VW)
    for hh in range(2):
        h = 2 * pr + hh
        nc.tensor.matmul(
            a_ps[hh],
            vv[:, h, :HD + 1],
            pt[:, hh * CH:(hh + 1) * CH],
            start=start, stop=stop)


_NC_CACHE = {}


def build(enable_asserts=False):
    key = enable_asserts
    if key not in _NC_CACHE:
        nc = bacc.Bacc("TRN2", target_bir_lowering=False, debug=False,
                       enable_asserts=enable_asserts, num_devices=NCORES)
        build_program(nc)
        nc.compile()
        _NC_CACHE[key] = nc
    return _NC_CACHE[key]


def round_f32r(a):
    """Round f32 array to fp32r (tfloat32: 11 explicit mantissa bits, RNE)."""
    u = np.ascontiguousarray(np.asarray(a, np.float32)).view(np.uint32)
    r = (u.astype(np.uint64) + 0x7FF + ((u >> 12) & 1)).astype(np.uint32)
    return (r & np.uint32(0xFFFFF000)).view(np.float32)


def shard_inputs(x, Wq, Wk, Wv, Wo):
    x = round_f32r(x)
    Wq, Wk, Wv, Wo = (round_f32r(w) for w in (Wq, Wk, Wv, Wo))
    in_maps = []
    for c in range(NCORES):
        tp, dp = c % TP, c // TP
        cols = slice(DHC * tp, DHC * (tp + 1))
        xTc = np.ascontiguousarray(
            np.transpose(x[NB * dp:NB * (dp + 1)], (0, 2, 1)))
        in_maps.append({
            "xT": xTc,
            "wq": np.ascontiguousarray(Wq[:, cols]),
            "wk": np.ascontiguousarray(Wk[:, cols]),
            "wv": np.ascontiguousarray(Wv[:, cols]),
            "wo": np.ascontiguousarray(Wo[cols, :]),
        })
    return in_maps


def unshard_output(results, bo):
    out = np.zeros((B, S, D), np.float32)
    for c in range(NCORES):
        dp = c // TP
        o = results[c]["outT"]  # [NB, D, S]
        for i in range(NB):
            out[NB * dp + i] += o[i].T
    out += np.asarray(bo, np.float32)[None, None, :]
    return out


def kernel(x, Wq, Wk, Wv, Wo, bo, _trace=False):
    nc = build()
    in_maps = shard_inputs(x, Wq, Wk, Wv, Wo)
    res = run_bass_kernel_spmd(nc, in_maps, core_ids=list(range(NCORES)),
                               trace=_trace)
    out = unshard_output(res.results, bo)
    if _trace:
        return out, res
    return out
